# revision 11
# baseline (speedup 1.0000x reference)
"""Trainium2 Bass kernel for nn_MultiHeadAttention_5334349382389.

Sharding: 8 cores = 4 batches x 2 head-groups (4 heads each).
Core c handles batch b = c // 2, head-group g = c % 2 (heads 4g..4g+3).

Per-core math (matmuls in fp16 by default, fp32 PSUM accumulate):
  qhT = (Wq_g/8) @ x_b^T + bq_g/8        [256, 1024]   (score scale folded into Wq)
  khT = Wk_g @ x_b^T + bk_g              [256, 1024]
  vh  = x_b @ Wv_g^T                     [1024, 256]   (bv folded into host-side bias)
  per head h: scoresT[k,q] = K=64 contraction of khT/qhT, plus I @ edgeT in
      fp32r (edgeT is zeros on non-edge cores; Wq/bq head-0 slice zeroed on
      edge cores, so edge cores get scoresT == edgeT exactly)
  expT = exp(scoresT)                    (no max-subtraction; inputs are bounded)
  outT_raw[d,q] accum over k-tiles with lhsT = [vh | ones] -> row 64 = softmax denom
  OT = outT_raw[:64] * bcast(1/denom)
  partial = OT^T-contraction @ WoT_g     [1024, 512]

Host: transposes/slices/casts inputs per core, gathers
  out[b] = partial(b,0) + partial(b,1) + (bo + Wo @ bv)
(the bv term is exact because softmax rows sum to 1).
"""

import os
import sys

sys.path.insert(0, "/opt/trn_rl_repo")

import numpy as np

B, SEQ, DIN, DO = 4, 1024, 512, 512
NH_ALL, DK = 8, 64
NHC = 4            # heads per core
DH = NHC * DK      # 256 per-core projected dims
P = 128
CD = DIN // P      # 4 contraction chunks for projections
CH = DH // P       # 2 dh chunks
KT = SEQ // P      # 8 k-tiles
STR = 512          # q-stripe (matmul free dim)
NS = SEQ // STR    # 2 stripes
TVW = NHC * (DK + 1) + DK - 1  # 323: per-k-tile aux width (4x65 + 63 pad)

COMPUTE = os.environ.get("KERNEL_COMPUTE_DT", "fp16")  # fp16 | bf16 | fp32r

_nc = None


def _np_dt():
    import ml_dtypes

    return {
        "fp16": np.float16,
        "bf16": ml_dtypes.bfloat16,
        "fp32r": np.float32,
    }[COMPUTE]


def _build():
    global _nc
    if _nc is not None:
        return _nc
    import concourse.bacc as bacc
    import concourse.bass as bass
    import concourse.mybir as mybir
    import concourse.tile as tile

    f32 = mybir.dt.float32
    f32r = mybir.dt.float32r
    cdt = {
        "fp16": mybir.dt.float16,
        "bf16": mybir.dt.bfloat16,
        "fp32r": f32r,
    }[COMPUTE]
    Exp = mybir.ActivationFunctionType.Exp

    nc = bacc.Bacc("TRN2", target_bir_lowering=False, debug=False)

    xq = nc.dram_tensor("xq", (DIN, SEQ), cdt, kind="ExternalInput")
    xk = nc.dram_tensor("xk", (DIN, SEQ), cdt, kind="ExternalInput")
    xv = nc.dram_tensor("xv", (DIN, SEQ), cdt, kind="ExternalInput")
    wq = nc.dram_tensor("wq", (DIN, DH), cdt, kind="ExternalInput")
    wk = nc.dram_tensor("wk", (DIN, DH), cdt, kind="ExternalInput")
    wv = nc.dram_tensor("wv", (DIN, DH), cdt, kind="ExternalInput")
    wo = nc.dram_tensor("wo", (DH, DO), cdt, kind="ExternalInput")
    bq = nc.dram_tensor("bq", (DH, 1), f32, kind="ExternalInput")
    bk = nc.dram_tensor("bk", (DH, 1), f32, kind="ExternalInput")
    eye = nc.dram_tensor("eye", (P, P), cdt, kind="ExternalInput")
    vaux = nc.dram_tensor("vaux", (P, KT, TVW), cdt, kind="ExternalInput")
    kz = nc.dram_tensor("kz", (P, NHC * SEQ), cdt, kind="ExternalInput")
    edge = nc.dram_tensor("edge", (SEQ, SEQ), cdt, kind="ExternalInput")
    outp = nc.dram_tensor("outp", (SEQ, DO), cdt, kind="ExternalOutput")

    xq_r = xq.rearrange("(c p) n -> c p n", p=P)
    xk_r = xk.rearrange("(c p) n -> c p n", p=P)
    xv_r = xv.rearrange("(c p) n -> c p n", p=P)
    edge_r = edge.rearrange("(t p) n -> t p n", p=P)
    outp_r = outp.rearrange("(t p) n -> t p n", p=P)

    def sl(s):
        return slice(s * STR, (s + 1) * STR)

    with tile.TileContext(nc) as tc:
        with (
            tc.tile_pool(name="inp", bufs=1) as inp,
            tc.tile_pool(name="wts", bufs=1) as wts,
            tc.tile_pool(name="qkp", bufs=1) as qkp,
            tc.tile_pool(name="vhap", bufs=1) as vhap,
            tc.tile_pool(name="expp", bufs=8) as expp,
            tc.tile_pool(name="otp", bufs=1) as otp,
            tc.tile_pool(name="rrp", bufs=2) as rrp,
            tc.tile_pool(name="rbp", bufs=2) as rbp,
            tc.tile_pool(name="outsp", bufs=3) as outsp,
            tc.tile_pool(name="edgp", bufs=3) as edgp,
            tc.tile_pool(name="bigp", bufs=2, space=bass.MemorySpace.PSUM) as bigp,
            tc.tile_pool(name="povp", bufs=2, space=bass.MemorySpace.PSUM) as povp,
        ):
            # ------- input loads: spread across sync/scalar/gpsimd rings -------
            # Ordered so the q-projection's first chunk can start ASAP.
            txq = inp.tile([P, CD, SEQ], cdt, tag="txq")
            txk = inp.tile([P, CD, SEQ], cdt, tag="txk")
            txv = inp.tile([P, CD, SEQ], cdt, tag="txv")
            twq = wts.tile([P, CD, DH], cdt, tag="twq")
            twk = wts.tile([P, CD, DH], cdt, tag="twk")
            twv = wts.tile([P, CD, DH], cdt, tag="twv")
            two = wts.tile([P, CH, DO], cdt, tag="two")
            tbq = wts.tile([P, CH, 1], f32, tag="tbq")
            tbk = wts.tile([P, CH, 1], f32, tag="tbk")
            teye = wts.tile([P, P], cdt, tag="teye")
            tvha = vhap.tile([P, KT, TVW], cdt, tag="tvha")

            nc.sync.dma_start(out=teye, in_=eye[:])
            nc.sync.dma_start(out=twq, in_=wq.rearrange("(c p) d -> p c d", p=P))
            for cd in range(CD):
                nc.sync.dma_start(out=txq[:, cd, :], in_=xq_r[cd])
                if cd == 1:
                    nc.sync.dma_start(
                        out=tbq, in_=bq.rearrange("(c p) o -> p c o", p=P)
                    )
            nc.scalar.dma_start(out=twk, in_=wk.rearrange("(c p) d -> p c d", p=P))
            for cd in range(CD):
                nc.scalar.dma_start(out=txk[:, cd, :], in_=xk_r[cd])
                if cd == 1:
                    nc.scalar.dma_start(
                        out=tbk, in_=bk.rearrange("(c p) o -> p c o", p=P)
                    )
            for cd in range(CD):
                nc.sync.dma_start(out=txv[:, cd, :], in_=xv_r[cd])
            nc.gpsimd.dma_start(out=twv, in_=wv.rearrange("(c p) d -> p c d", p=P))
            nc.gpsimd.dma_start(out=tvha, in_=vaux[:])
            nc.scalar.dma_start(out=two, in_=wo.rearrange("(c p) d -> p c d", p=P))

            # PE warmup: junk matmuls on the early-arriving identity tile keep
            # the HAM activity window busy while the input DMAs land, so the
            # projection starts at the 2.4 GHz clock.
            wu = bigp.tile([P, SEQ], f32, tag="big")
            for i in range(60):
                nc.tensor.matmul(
                    wu[:, 0:P], lhsT=teye[:], rhs=teye[:], start=True, stop=True
                )

            # ---------------- projections ----------------
            tqh = qkp.tile([P, CH, SEQ], cdt, tag="tqh")
            khp = qkp.tile([P, NHC, SEQ], cdt, tag="khp")
            # zero the unused partition-halves of khp (even heads: parts 64-127,
            # odd heads: parts 0-63) so K=128 score matmuls see zero weights there
            kz_r = kz.rearrange("p (h n) -> p h n", n=SEQ)
            nc.sync.dma_start(out=khp[0:DK, 1 :: 2, :], in_=kz_r[0:DK, 0:2, :])
            nc.sync.dma_start(out=khp[DK:P, 0 :: 2, :], in_=kz_r[DK:P, 0:2, :])
            def proj_q(ch):
                pt = bigp.tile([P, SEQ], f32, tag="big")
                for cd in range(CD):
                    for s in range(NS):
                        nc.tensor.matmul(
                            pt[:, sl(s)],
                            lhsT=twq[:, cd, ch * P : (ch + 1) * P],
                            rhs=txq[:, cd, sl(s)],
                            start=(cd == 0),
                            stop=(cd == CD - 1),
                        )
                nc.vector.tensor_scalar_add(
                    out=tqh[:, ch, :], in0=pt[:], scalar1=tbq[:, ch, :]
                )

            def proj_k(ch):
                pt = bigp.tile([P, SEQ], f32, tag="big")
                for cd in range(CD):
                    for s in range(NS):
                        nc.tensor.matmul(
                            pt[:, sl(s)],
                            lhsT=twk[:, cd, ch * P : (ch + 1) * P],
                            rhs=txk[:, cd, sl(s)],
                            start=(cd == 0),
                            stop=(cd == CD - 1),
                        )
                nc.vector.tensor_scalar_add(
                    out=khp[0:DK, 2 * ch, :],
                    in0=pt[0:DK, :],
                    scalar1=tbk[0:DK, ch, :],
                )
                nc.vector.tensor_scalar_add(
                    out=khp[DK:P, 2 * ch + 1, :],
                    in0=pt[DK:P, :],
                    scalar1=tbk[DK:P, ch, :],
                )

            proj_q(0)
            proj_k(0)

            # v: [s, dh] tiles written into vh_aug (65-wide per head, col 64 = 1.0)
            for st in range(KT):
                pt = bigp.tile([P, SEQ], f32, tag="big")
                for cd in range(CD):
                    nc.tensor.matmul(
                        pt[:, 0:DH],
                        lhsT=txv[:, cd, st * P : (st + 1) * P],
                        rhs=twv[:, cd, :],
                        start=(cd == 0),
                        stop=(cd == CD - 1),
                    )
                nc.vector.tensor_copy(
                    out=tvha[:, st, 0 : NHC * (DK + 1)].rearrange(
                        "p (h w) -> p h w", w=DK + 1
                    )[:, :, 0:DK],
                    in_=pt[:, 0:DH].rearrange("p (h d) -> p h d", h=NHC),
                )
            proj_q(1)
            proj_k(1)

            # ---------------- attention per head ----------------
            tot = otp.tile([P, CH, SEQ], cdt, tag="tot")
            for h in range(NHC):
                ch, off = h // 2, (h % 2) * DK
                pv = povp.tile([P, SEQ], f32, tag="pov")
                for kt in range(KT):
                    stt = bigp.tile([P, SEQ], f32, tag="big")
                    ed = None
                    if h == 0:
                        ed = edgp.tile([P, SEQ], cdt, tag="edg")
                        nc.gpsimd.dma_start(out=ed, in_=edge_r[kt])
                    for s in range(NS):
                        nc.tensor.matmul(
                            stt[:, sl(s)],
                            lhsT=khp[:, h, kt * P : (kt + 1) * P],
                            rhs=tqh[:, ch, sl(s)],
                            start=True,
                            stop=(h != 0),
                        )
                        if h == 0:
                            nc.tensor.matmul(
                                stt[:, sl(s)],
                                lhsT=teye[:],
                                rhs=ed[:, sl(s)],
                                start=False,
                                stop=True,
                            )
                    te = expp.tile([P, SEQ], cdt, tag="expT")
                    nc.scalar.activation(out=te, in_=stt[:], func=Exp)
                    for s in range(NS):
                        nc.tensor.matmul(
                            pv[:, sl(s)],
                            lhsT=tvha[:, kt, h * (DK + 1) : h * (DK + 1) + P],
                            rhs=te[:, sl(s)],
                            start=(kt == 0),
                            stop=(kt == KT - 1),
                        )
                rr = rrp.tile([1, SEQ], f32, tag="rr")
                rs = rrp.tile([1, SEQ], f32, tag="rs")
                nc.vector.tensor_copy(out=rs[:], in_=pv[DK : DK + 1, :])
                nc.vector.reciprocal_approx_fast(out=rr[:], in_=rs[:])
                rb = rbp.tile([DK, SEQ], f32, tag="rb")
                nc.gpsimd.partition_broadcast(rb[:], rr[:])
                nc.vector.tensor_mul(tot[off : off + DK, ch, :], pv[0:DK, :], rb[:])

            # ---------------- output projection ----------------
            for m in range(KT):
                po = bigp.tile([P, SEQ], f32, tag="big")
                for ch in range(CH):
                    nc.tensor.matmul(
                        po[:, 0:DO],
                        lhsT=tot[:, ch, m * P : (m + 1) * P],
                        rhs=two[:, ch, :],
                        start=(ch == 0),
                        stop=(ch == CH - 1),
                    )
                ots = outsp.tile([P, DO], cdt, tag="outs")
                nc.vector.tensor_copy(out=ots, in_=po[:, 0:DO])
                nc.sync.dma_start(out=outp_r[m], in_=ots)

    nc.compile()
    _nc = nc
    return nc


def _in_maps(q, k, v, edge_matrix, Wq, bq, Wk, bk, Wv, Wo):
    dt = _np_dt()
    zeros_edge = np.zeros((SEQ, SEQ), dt)
    edge_t = np.ascontiguousarray(edge_matrix.T).astype(dt)
    ident = np.eye(P, dtype=dt)
    vaux_in = np.zeros((P, KT, TVW), dt)
    for h in range(NHC):
        vaux_in[:, :, h * (DK + 1) + DK] = 1.0
    kz_in = np.zeros((P, NHC * SEQ), dt)
    xt = {}
    for b in range(B):
        xt[b] = (
            np.ascontiguousarray(q[b].T).astype(dt),
            np.ascontiguousarray(k[b].T).astype(dt),
            np.ascontiguousarray(v[b].T).astype(dt),
        )
    maps = []
    for c in range(8):
        b, g = c // 2, c % 2
        is_edge = g == 0 and b < 2
        rows = slice(g * DH, (g + 1) * DH)
        wq_c = np.ascontiguousarray(Wq[rows].T) * np.float32(1.0 / 8.0)
        bq_c = (bq[rows] * np.float32(1.0 / 8.0)).reshape(DH, 1).copy()
        if is_edge:
            wq_c[:, 0:DK] = 0.0
            bq_c[0:DK] = 0.0
        maps.append(
            {
                "xq": xt[b][0],
                "xk": xt[b][1],
                "xv": xt[b][2],
                "wq": wq_c.astype(dt),
                "wk": np.ascontiguousarray(Wk[rows].T).astype(dt),
                "wv": np.ascontiguousarray(Wv[rows].T).astype(dt),
                "wo": np.ascontiguousarray(Wo[:, rows].T).astype(dt),
                "bq": bq_c,
                "bk": bk[rows].reshape(DH, 1).copy(),
                "eye": ident,
                "vaux": vaux_in,
                "kz": kz_in,
                "edge": edge_t if is_edge else zeros_edge,
            }
        )
    return maps


def _ensure_ntff_hook():
    """Register the axon NTFF profile hook if the image's antenv lacks it."""
    import contextlib
    import ctypes
    import types

    try:
        from antenv.axon_hooks import get_axon_ntff_profile_hook  # noqa: F401
        return
    except ImportError:
        pass

    so_path = "/opt/axon/libaxon_pjrt.so"
    try:
        lib = ctypes.CDLL(so_path)
    except OSError:
        return
    if not hasattr(lib, "axon_start_nrt_profile"):
        return
    lib.axon_start_nrt_profile.argtypes = [
        ctypes.POINTER(ctypes.c_int64),
        ctypes.c_size_t,
    ]
    lib.axon_start_nrt_profile.restype = ctypes.c_int64
    lib.axon_stop_nrt_profile.argtypes = [ctypes.c_char_p]
    lib.axon_stop_nrt_profile.restype = ctypes.c_int64

    @contextlib.contextmanager
    def _hook(output_dir, device_ids):
        import jax

        jax.devices()
        if device_ids:
            ids = (ctypes.c_int64 * len(device_ids))(*device_ids)
            rc = lib.axon_start_nrt_profile(ids, len(device_ids))
        else:
            rc = lib.axon_start_nrt_profile(None, 0)
        if rc != 0:
            raise RuntimeError(f"axon_start_nrt_profile rc={rc}")
        try:
            yield
        finally:
            n = lib.axon_stop_nrt_profile(str(output_dir).encode())
            if n < 0:
                raise RuntimeError(f"axon_stop_nrt_profile rc={n}")

    _state = {"hook": _hook}
    mod = types.ModuleType("antenv.axon_hooks")
    mod.get_axon_ntff_profile_hook = lambda: _state["hook"]
    mod.set_axon_ntff_profile_hook = lambda h: _state.__setitem__("hook", h)
    import antenv

    antenv.axon_hooks = mod
    sys.modules["antenv.axon_hooks"] = mod


def kernel(q, k, v, edge_matrix, Wq, bq, Wk, bk, Wv, bv, Wo, bo, _trace=False):
    from concourse.bass_utils import run_bass_kernel_spmd

    if _trace:
        _ensure_ntff_hook()

    q, k, v = (np.asarray(t, np.float32) for t in (q, k, v))
    edge_matrix = np.asarray(edge_matrix, np.float32)
    Wq, bq, Wk, bk, Wv, bv, Wo, bo = (
        np.asarray(t, np.float32) for t in (Wq, bq, Wk, bk, Wv, bv, Wo, bo)
    )

    nc = _build()
    maps = _in_maps(q, k, v, edge_matrix, Wq, bq, Wk, bk, Wv, Wo)
    res = run_bass_kernel_spmd(nc, maps, core_ids=list(range(8)), trace=_trace)

    bo_eff = bo + Wo @ bv
    out = np.empty((B, SEQ, DO), np.float32)
    for b in range(B):
        out[b] = res.results[2 * b]["outp"] + res.results[2 * b + 1]["outp"] + bo_eff
    if _trace:
        return out, res
    return out


# revision 12
# speedup vs baseline: 1.0364x; 1.0364x over previous
"""Trainium2 Bass kernel for nn_MultiHeadAttention_5334349382389.

Sharding: 8 cores = 4 batches x 2 head-groups (4 heads each).
Core c handles batch b = c // 2, head-group g = c % 2 (heads 4g..4g+3).

Per-core math (matmuls in fp16 by default, fp32 PSUM accumulate):
  qhT = (Wq_g/8) @ x_b^T + bq_g/8        [256, 1024]   (score scale folded into Wq)
  khT = Wk_g @ x_b^T + bk_g              [256, 1024]
  vh  = x_b @ Wv_g^T                     [1024, 256]   (bv folded into host-side bias)
  per head h: scoresT[k,q] = K=64 contraction of khT/qhT, plus I @ edgeT in
      fp32r (edgeT is zeros on non-edge cores; Wq/bq head-0 slice zeroed on
      edge cores, so edge cores get scoresT == edgeT exactly)
  expT = exp(scoresT)                    (no max-subtraction; inputs are bounded)
  outT_raw[d,q] accum over k-tiles with lhsT = [vh | ones] -> row 64 = softmax denom
  OT = outT_raw[:64] * bcast(1/denom)
  partial = OT^T-contraction @ WoT_g     [1024, 512]

Host: transposes/slices/casts inputs per core, gathers
  out[b] = partial(b,0) + partial(b,1) + (bo + Wo @ bv)
(the bv term is exact because softmax rows sum to 1).
"""

import os
import sys

sys.path.insert(0, "/opt/trn_rl_repo")

import numpy as np

B, SEQ, DIN, DO = 4, 1024, 512, 512
NH_ALL, DK = 8, 64
NHC = 4            # heads per core
DH = NHC * DK      # 256 per-core projected dims
P = 128
CD = DIN // P      # 4 contraction chunks for projections
CH = DH // P       # 2 dh chunks
KT = SEQ // P      # 8 k-tiles
STR = 512          # q-stripe (matmul free dim)
NS = SEQ // STR    # 2 stripes
TVW = NHC * (DK + 1) + DK - 1  # 323: per-k-tile aux width (4x65 + 63 pad)

COMPUTE = os.environ.get("KERNEL_COMPUTE_DT", "fp16")  # fp16 | bf16 | fp32r

_nc = None


def _np_dt():
    import ml_dtypes

    return {
        "fp16": np.float16,
        "bf16": ml_dtypes.bfloat16,
        "fp32r": np.float32,
    }[COMPUTE]


def _build():
    global _nc
    if _nc is not None:
        return _nc
    import concourse.bacc as bacc
    import concourse.bass as bass
    import concourse.mybir as mybir
    import concourse.tile as tile

    f32 = mybir.dt.float32
    f32r = mybir.dt.float32r
    cdt = {
        "fp16": mybir.dt.float16,
        "bf16": mybir.dt.bfloat16,
        "fp32r": f32r,
    }[COMPUTE]
    Exp = mybir.ActivationFunctionType.Exp

    nc = bacc.Bacc("TRN2", target_bir_lowering=False, debug=False)

    xq = nc.dram_tensor("xq", (DIN, SEQ), cdt, kind="ExternalInput")
    xk = nc.dram_tensor("xk", (DIN, SEQ), cdt, kind="ExternalInput")
    xv = nc.dram_tensor("xv", (DIN, SEQ), cdt, kind="ExternalInput")
    wq = nc.dram_tensor("wq", (DIN, DH), cdt, kind="ExternalInput")
    wk = nc.dram_tensor("wk", (DIN, DH), cdt, kind="ExternalInput")
    wv = nc.dram_tensor("wv", (DIN, DH), cdt, kind="ExternalInput")
    wo = nc.dram_tensor("wo", (DH, DO), cdt, kind="ExternalInput")
    bq = nc.dram_tensor("bq", (DH, 1), f32, kind="ExternalInput")
    bk = nc.dram_tensor("bk", (DH, 1), f32, kind="ExternalInput")
    eye = nc.dram_tensor("eye", (P, P), cdt, kind="ExternalInput")
    vaux = nc.dram_tensor("vaux", (P, KT, TVW), cdt, kind="ExternalInput")
    edge = nc.dram_tensor("edge", (SEQ, SEQ), cdt, kind="ExternalInput")
    outp = nc.dram_tensor("outp", (SEQ, DO), cdt, kind="ExternalOutput")

    xq_r = xq.rearrange("(c p) n -> c p n", p=P)
    xk_r = xk.rearrange("(c p) n -> c p n", p=P)
    xv_r = xv.rearrange("(c p) n -> c p n", p=P)
    edge_r = edge.rearrange("(t p) n -> t p n", p=P)
    outp_r = outp.rearrange("(t p) n -> t p n", p=P)

    def sl(s):
        return slice(s * STR, (s + 1) * STR)

    with tile.TileContext(nc) as tc:
        with (
            tc.tile_pool(name="inp", bufs=1) as inp,
            tc.tile_pool(name="wts", bufs=1) as wts,
            tc.tile_pool(name="qkp", bufs=1) as qkp,
            tc.tile_pool(name="vhap", bufs=1) as vhap,
            tc.tile_pool(name="expp", bufs=8) as expp,
            tc.tile_pool(name="otp", bufs=1) as otp,
            tc.tile_pool(name="rrp", bufs=2) as rrp,
            tc.tile_pool(name="rbp", bufs=2) as rbp,
            tc.tile_pool(name="outsp", bufs=3) as outsp,
            tc.tile_pool(name="edgp", bufs=3) as edgp,
            tc.tile_pool(name="bigp", bufs=2, space=bass.MemorySpace.PSUM) as bigp,
            tc.tile_pool(name="povp", bufs=2, space=bass.MemorySpace.PSUM) as povp,
        ):
            # ------- input loads: spread across sync/scalar/gpsimd rings -------
            # Ordered so the q-projection's first chunk can start ASAP.
            txq = inp.tile([P, CD, SEQ], cdt, tag="txq")
            txk = inp.tile([P, CD, SEQ], cdt, tag="txk")
            txv = inp.tile([P, CD, SEQ], cdt, tag="txv")
            twq = wts.tile([P, CD, DH], cdt, tag="twq")
            twk = wts.tile([P, CD, DH], cdt, tag="twk")
            twv = wts.tile([P, CD, DH], cdt, tag="twv")
            two = wts.tile([P, CH, DO], cdt, tag="two")
            tbq = wts.tile([P, CH, 1], f32, tag="tbq")
            tbk = wts.tile([P, CH, 1], f32, tag="tbk")
            teye = wts.tile([P, P], cdt, tag="teye")
            tvha = vhap.tile([P, KT, TVW], cdt, tag="tvha")

            nc.sync.dma_start(out=teye, in_=eye[:])
            nc.sync.dma_start(out=twq, in_=wq.rearrange("(c p) d -> p c d", p=P))
            for cd in range(CD):
                nc.sync.dma_start(out=txq[:, cd, :], in_=xq_r[cd])
                if cd == 1:
                    nc.sync.dma_start(
                        out=tbq, in_=bq.rearrange("(c p) o -> p c o", p=P)
                    )
            nc.scalar.dma_start(out=twk, in_=wk.rearrange("(c p) d -> p c d", p=P))
            for cd in range(CD):
                nc.scalar.dma_start(out=txk[:, cd, :], in_=xk_r[cd])
                if cd == 1:
                    nc.scalar.dma_start(
                        out=tbk, in_=bk.rearrange("(c p) o -> p c o", p=P)
                    )
            nc.gpsimd.dma_start(out=twv, in_=wv.rearrange("(c p) d -> p c d", p=P))
            nc.gpsimd.dma_start(out=tvha, in_=vaux[:])
            for cd in range(CD):
                nc.gpsimd.dma_start(out=txv[:, cd, :], in_=xv_r[cd])
            nc.scalar.dma_start(out=two, in_=wo.rearrange("(c p) d -> p c d", p=P))

            # PE warmup: junk matmuls on the early-arriving identity tile keep
            # the HAM activity window busy while the input DMAs land, so the
            # projection starts at the 2.4 GHz clock.
            wu = bigp.tile([P, SEQ], f32, tag="big")
            for i in range(60):
                nc.tensor.matmul(
                    wu[:, 0:P], lhsT=teye[:], rhs=teye[:], start=True, stop=True
                )

            # ---------------- projections ----------------
            tqh = qkp.tile([P, CH, SEQ], cdt, tag="tqh")
            khp = qkp.tile([P, NHC, SEQ], cdt, tag="khp")
            # zero the unused partition-halves of khp (even heads: parts 64-127,
            # odd heads: parts 0-63) so K=128 score matmuls see zero weights there
            def proj_q(ch):
                pt = bigp.tile([P, SEQ], f32, tag="big")
                for cd in range(CD):
                    for s in range(NS):
                        nc.tensor.matmul(
                            pt[:, sl(s)],
                            lhsT=twq[:, cd, ch * P : (ch + 1) * P],
                            rhs=txq[:, cd, sl(s)],
                            start=(cd == 0),
                            stop=(cd == CD - 1),
                        )
                nc.vector.tensor_scalar_add(
                    out=tqh[:, ch, :], in0=pt[:], scalar1=tbq[:, ch, :]
                )

            def proj_k(ch):
                pt = bigp.tile([P, SEQ], f32, tag="big")
                for cd in range(CD):
                    for s in range(NS):
                        nc.tensor.matmul(
                            pt[:, sl(s)],
                            lhsT=twk[:, cd, ch * P : (ch + 1) * P],
                            rhs=txk[:, cd, sl(s)],
                            start=(cd == 0),
                            stop=(cd == CD - 1),
                        )
                nc.vector.tensor_scalar_add(
                    out=khp[0:DK, 2 * ch, :],
                    in0=pt[0:DK, :],
                    scalar1=tbk[0:DK, ch, :],
                )
                nc.vector.tensor_scalar_add(
                    out=khp[DK:P, 2 * ch + 1, :],
                    in0=pt[DK:P, :],
                    scalar1=tbk[DK:P, ch, :],
                )
                nc.vector.tensor_scalar_mul(khp[DK:P, 2 * ch, :], pt[DK:P, :], 0.0)
                nc.vector.tensor_scalar_mul(khp[0:DK, 2 * ch + 1, :], pt[0:DK, :], 0.0)

            proj_q(0)
            proj_k(0)

            # v: [s, dh] tiles written into vh_aug (65-wide per head, col 64 = 1.0)
            for st in range(KT):
                pt = bigp.tile([P, SEQ], f32, tag="big")
                for cd in range(CD):
                    nc.tensor.matmul(
                        pt[:, 0:DH],
                        lhsT=txv[:, cd, st * P : (st + 1) * P],
                        rhs=twv[:, cd, :],
                        start=(cd == 0),
                        stop=(cd == CD - 1),
                    )
                nc.vector.tensor_copy(
                    out=tvha[:, st, 0 : NHC * (DK + 1)].rearrange(
                        "p (h w) -> p h w", w=DK + 1
                    )[:, :, 0:DK],
                    in_=pt[:, 0:DH].rearrange("p (h d) -> p h d", h=NHC),
                )
            proj_q(1)
            proj_k(1)

            # ---------------- attention per head ----------------
            tot = otp.tile([P, CH, SEQ], cdt, tag="tot")
            for h in (1, 2, 3, 0):
                ch, off = h // 2, (h % 2) * DK
                pv = povp.tile([P, SEQ], f32, tag="pov")
                for kt in range(KT):
                    stt = bigp.tile([P, SEQ], f32, tag="big")
                    ed = None
                    if h == 0:
                        ed = edgp.tile([P, SEQ], cdt, tag="edg")
                        nc.gpsimd.dma_start(out=ed, in_=edge_r[kt])
                    for s in range(NS):
                        nc.tensor.matmul(
                            stt[:, sl(s)],
                            lhsT=khp[:, h, kt * P : (kt + 1) * P],
                            rhs=tqh[:, ch, sl(s)],
                            start=True,
                            stop=(h != 0),
                        )
                        if h == 0:
                            nc.tensor.matmul(
                                stt[:, sl(s)],
                                lhsT=teye[:],
                                rhs=ed[:, sl(s)],
                                start=False,
                                stop=True,
                            )
                    te = expp.tile([P, SEQ], cdt, tag="expT")
                    nc.scalar.activation(out=te, in_=stt[:], func=Exp)
                    for s in range(NS):
                        nc.tensor.matmul(
                            pv[:, sl(s)],
                            lhsT=tvha[:, kt, h * (DK + 1) : h * (DK + 1) + P],
                            rhs=te[:, sl(s)],
                            start=(kt == 0),
                            stop=(kt == KT - 1),
                        )
                rr = rrp.tile([1, SEQ], f32, tag="rr")
                rs = rrp.tile([1, SEQ], f32, tag="rs")
                nc.vector.tensor_copy(out=rs[:], in_=pv[DK : DK + 1, :])
                nc.vector.reciprocal_approx_fast(out=rr[:], in_=rs[:])
                rb = rbp.tile([DK, SEQ], f32, tag="rb")
                nc.gpsimd.partition_broadcast(rb[:], rr[:])
                nc.vector.tensor_mul(tot[off : off + DK, ch, :], pv[0:DK, :], rb[:])

            # ---------------- output projection ----------------
            for m in range(KT):
                po = bigp.tile([P, SEQ], f32, tag="big")
                for i, ch in enumerate((1, 0)):
                    nc.tensor.matmul(
                        po[:, 0:DO],
                        lhsT=tot[:, ch, m * P : (m + 1) * P],
                        rhs=two[:, ch, :],
                        start=(i == 0),
                        stop=(i == CH - 1),
                    )
                ots = outsp.tile([P, DO], cdt, tag="outs")
                nc.vector.tensor_copy(out=ots, in_=po[:, 0:DO])
                nc.sync.dma_start(out=outp_r[m], in_=ots)

    nc.compile()
    _nc = nc
    return nc


def _in_maps(q, k, v, edge_matrix, Wq, bq, Wk, bk, Wv, Wo):
    dt = _np_dt()
    zeros_edge = np.zeros((SEQ, SEQ), dt)
    edge_t = np.ascontiguousarray(edge_matrix.T).astype(dt)
    ident = np.eye(P, dtype=dt)
    vaux_in = np.zeros((P, KT, TVW), dt)
    for h in range(NHC):
        vaux_in[:, :, h * (DK + 1) + DK] = 1.0
    xt = {}
    for b in range(B):
        xt[b] = (
            np.ascontiguousarray(q[b].T).astype(dt),
            np.ascontiguousarray(k[b].T).astype(dt),
            np.ascontiguousarray(v[b].T).astype(dt),
        )
    maps = []
    for c in range(8):
        b, g = c // 2, c % 2
        is_edge = g == 0 and b < 2
        rows = slice(g * DH, (g + 1) * DH)
        wq_c = np.ascontiguousarray(Wq[rows].T) * np.float32(1.0 / 8.0)
        bq_c = (bq[rows] * np.float32(1.0 / 8.0)).reshape(DH, 1).copy()
        if is_edge:
            wq_c[:, 0:DK] = 0.0
            bq_c[0:DK] = 0.0
        maps.append(
            {
                "xq": xt[b][0],
                "xk": xt[b][1],
                "xv": xt[b][2],
                "wq": wq_c.astype(dt),
                "wk": np.ascontiguousarray(Wk[rows].T).astype(dt),
                "wv": np.ascontiguousarray(Wv[rows].T).astype(dt),
                "wo": np.ascontiguousarray(Wo[:, rows].T).astype(dt),
                "bq": bq_c,
                "bk": bk[rows].reshape(DH, 1).copy(),
                "eye": ident,
                "vaux": vaux_in,
                "edge": edge_t if is_edge else zeros_edge,
            }
        )
    return maps


def _ensure_ntff_hook():
    """Register the axon NTFF profile hook if the image's antenv lacks it."""
    import contextlib
    import ctypes
    import types

    try:
        from antenv.axon_hooks import get_axon_ntff_profile_hook  # noqa: F401
        return
    except ImportError:
        pass

    so_path = "/opt/axon/libaxon_pjrt.so"
    try:
        lib = ctypes.CDLL(so_path)
    except OSError:
        return
    if not hasattr(lib, "axon_start_nrt_profile"):
        return
    lib.axon_start_nrt_profile.argtypes = [
        ctypes.POINTER(ctypes.c_int64),
        ctypes.c_size_t,
    ]
    lib.axon_start_nrt_profile.restype = ctypes.c_int64
    lib.axon_stop_nrt_profile.argtypes = [ctypes.c_char_p]
    lib.axon_stop_nrt_profile.restype = ctypes.c_int64

    @contextlib.contextmanager
    def _hook(output_dir, device_ids):
        import jax

        jax.devices()
        if device_ids:
            ids = (ctypes.c_int64 * len(device_ids))(*device_ids)
            rc = lib.axon_start_nrt_profile(ids, len(device_ids))
        else:
            rc = lib.axon_start_nrt_profile(None, 0)
        if rc != 0:
            raise RuntimeError(f"axon_start_nrt_profile rc={rc}")
        try:
            yield
        finally:
            n = lib.axon_stop_nrt_profile(str(output_dir).encode())
            if n < 0:
                raise RuntimeError(f"axon_stop_nrt_profile rc={n}")

    _state = {"hook": _hook}
    mod = types.ModuleType("antenv.axon_hooks")
    mod.get_axon_ntff_profile_hook = lambda: _state["hook"]
    mod.set_axon_ntff_profile_hook = lambda h: _state.__setitem__("hook", h)
    import antenv

    antenv.axon_hooks = mod
    sys.modules["antenv.axon_hooks"] = mod


def kernel(q, k, v, edge_matrix, Wq, bq, Wk, bk, Wv, bv, Wo, bo, _trace=False):
    from concourse.bass_utils import run_bass_kernel_spmd

    if _trace:
        _ensure_ntff_hook()

    q, k, v = (np.asarray(t, np.float32) for t in (q, k, v))
    edge_matrix = np.asarray(edge_matrix, np.float32)
    Wq, bq, Wk, bk, Wv, bv, Wo, bo = (
        np.asarray(t, np.float32) for t in (Wq, bq, Wk, bk, Wv, bv, Wo, bo)
    )

    nc = _build()
    maps = _in_maps(q, k, v, edge_matrix, Wq, bq, Wk, bk, Wv, Wo)
    res = run_bass_kernel_spmd(nc, maps, core_ids=list(range(8)), trace=_trace)

    bo_eff = bo + Wo @ bv
    out = np.empty((B, SEQ, DO), np.float32)
    for b in range(B):
        out[b] = res.results[2 * b]["outp"] + res.results[2 * b + 1]["outp"] + bo_eff
    if _trace:
        return out, res
    return out
